# revision 9
# baseline (speedup 1.0000x reference)
"""Trainium2 Bass kernel: GRU decoder with Luong attention (B=32, T=S=512, H=1024, D=80).

Strategy (8 NeuronCores, data-parallel over batch, 4 sequences per core):
  P0: gx = W_ih @ x precomputed for the n gate only (fp32r matmuls) -> DRAM.
      The r,z gate gx+bias are folded into the recurrent PSUM accumulation
      via an extra matmul per chunk against the augmented input x~=[x;1]
      (weights [W_ih_rz; b_rz], bf16, zero-padded to 128 partitions — the
      PE FWL weight loader corrupts sub-128-partition bf16 loads).
  P1: sequential GRU in transposed layout (H on partitions, batch on free).
      Per step: 16 h-independent gx matmuls (cover the previous step's gate
      tail), then 192 (LDWEIGHTS+MATMUL) pairs streaming W_hh.T bf16 tiles
      against the bf16 h vector. Gate sigmoids read PSUM directly (bias
      folded via x~), the bf16 h for the next step is written by DVE right
      after the z-multiply (same-engine FIFO), and the f32 h copy (catT,
      attention input) goes to GPSIMD off the critical path. Explicit
      scheduling deps pin the gx matmuls behind the previous step's last
      matmul and keep the ACT queue order [tanh, z-sigmoid] — the Tile
      scheduler's cost model underestimates the matmul stream and otherwise
      misorders both.
  P2: attention for all timesteps at once (scoresT -> softmax -> PE
      transpose -> ctxT), then the concat linear (tanh) and output linear
      as fp32r matmuls.

All per-core inputs are sliced/transposed on the host; the 8 cores run the
same NEFF via run_bass_kernel_spmd with per-core input maps.
"""

import os
import sys

for _p in ("/opt/trn_rl_repo", "/root/.axon_site/_ro/trn_rl_repo"):
    if os.path.isdir(_p) and _p not in sys.path:
        sys.path.insert(0, _p)

import numpy as np
import ml_dtypes

import concourse.bass as bass
import concourse.mybir as mybir
import concourse.tile as tile
from concourse import bacc
from concourse.bass_utils import run_bass_kernel_spmd
from concourse.masks import make_identity

dt = mybir.dt
AF = mybir.ActivationFunctionType
ALU = mybir.AluOpType

H, D, B, S, T = 1024, 80, 32, 512, 512
BL = 4          # batch per core
NCORES = 8
KC = 8          # H / 128
MC = 24         # 3H / 128
KC2 = 16        # 2H / 128
NC8 = 8         # n-gate chunk count


def _build():
    nc = bacc.Bacc("TRN2", target_bir_lowering=False, debug=False,
                   num_devices=NCORES)
    f32r = dt.float32r

    w_hhT = nc.dram_tensor("w_hhT", [H, 3 * H], dt.bfloat16, kind="ExternalInput")
    w_ihTn = nc.dram_tensor("w_ihTn", [D, 3 * H], dt.float32, kind="ExternalInput")
    xT = nc.dram_tensor("xT", [D, T * BL], dt.float32, kind="ExternalInput")
    bias_nT = nc.dram_tensor("bias_nT", [128, MC], dt.float32, kind="ExternalInput")
    h0T = nc.dram_tensor("h0T", [128, KC, BL], dt.float32, kind="ExternalInput")
    encT_d = nc.dram_tensor("encT_d", [KC, 128, BL, S], dt.float32, kind="ExternalInput")
    enc_d = nc.dram_tensor("enc_d", [4, 128, BL, H], dt.float32, kind="ExternalInput")
    w_cT = nc.dram_tensor("w_cT", [2 * H, H], dt.float32, kind="ExternalInput")
    b_cT = nc.dram_tensor("b_cT", [128, KC], dt.float32, kind="ExternalInput")
    w_oT = nc.dram_tensor("w_oT", [H, D], dt.float32, kind="ExternalInput")
    b_o_b = nc.dram_tensor("b_o_b", [128, D], dt.float32, kind="ExternalInput")
    maskTb = nc.dram_tensor("maskTb", [128, 4, BL], dt.float32, kind="ExternalInput")
    bhhn = nc.dram_tensor("bhhn", [128, KC, BL], dt.float32, kind="ExternalInput")

    out_l = nc.dram_tensor("out_l", [BL, T, D], dt.float32, kind="ExternalOutput")
    gxT_d = nc.dram_tensor("gxT_d", [MC, 128, T * BL], dt.float32)

    with tile.TileContext(nc) as tc:
        with tc.tile_pool(name="persist", bufs=1) as persist:
            catT = persist.tile([128, KC2, T, BL], dt.float32r)
            ident_f = persist.tile([128, 128], dt.float32)
            make_identity(nc, ident_f)
            ident = persist.tile([128, 128], dt.float32r)
            nc.vector.tensor_copy(out=ident[:], in_=ident_f[:])

            # ---- P0: gx precompute (all three gates) ----
            with tc.tile_pool(name="p0", bufs=1) as p0, \
                 tc.tile_pool(name="p0o", bufs=4) as p0o, \
                 tc.tile_pool(name="psA", bufs=2, space="PSUM") as psA:
                biasn_sb = p0.tile([128, MC], dt.float32)
                nc.sync.dma_start(out=biasn_sb, in_=bias_nT.ap())
                xT_sb = p0.tile([D, T * BL], dt.float32r)
                nc.sync.dma_start(out=xT_sb, in_=xT.ap().bitcast(f32r))
                wihn_sb = p0.tile([D, MC, 128], dt.float32r)
                nc.sync.dma_start(
                    out=wihn_sb,
                    in_=w_ihTn.ap().bitcast(f32r).rearrange("p (mc m) -> p mc m", m=128))
                for mc in range(MC):
                    for nt in range(4):
                        ps = psA.tile([128, 512], dt.float32, tag="gx")
                        nc.tensor.matmul(ps[:], wihn_sb[:, mc, :],
                                         xT_sb[:, nt * 512:(nt + 1) * 512],
                                         start=True, stop=True)
                        gxs = p0o.tile([128, 512], dt.float32, tag="gxo")
                        if (mc + nt) % 2 == 0:
                            nc.scalar.activation(out=gxs[:], in_=ps[:], func=AF.Identity,
                                                 bias=biasn_sb[:, mc:mc + 1], scale=1.0)
                        else:
                            nc.vector.tensor_scalar_add(gxs[:], ps[:],
                                                        biasn_sb[:, mc:mc + 1])
                        nc.sync.dma_start(out=gxT_d.ap()[mc, :, nt * 512:(nt + 1) * 512],
                                          in_=gxs[:])

            # ---- P1: sequential GRU ----
            with tc.tile_pool(name="p1w", bufs=1) as p1w, \
                 tc.tile_pool(name="p1", bufs=1) as p1, \
                 tc.tile_pool(name="gxc", bufs=2) as gxcp, \
                 tc.tile_pool(name="p1t", bufs=3) as p1t, \
                 tc.tile_pool(name="psG", bufs=2, space="PSUM") as psG:
                w_sb = p1w.tile([128, KC, MC, 128], dt.bfloat16)
                for kc in range(KC):
                    nc.sync.dma_start(
                        out=w_sb[:, kc, :, :],
                        in_=w_hhT.ap()[kc * 128:(kc + 1) * 128, :]
                            .rearrange("p (mc m) -> p mc m", m=128))
                h_bf = p1.tile([128, 2, KC, BL], dt.bfloat16)
                h0_sb = p1.tile([128, KC, BL], dt.float32)
                nc.sync.dma_start(out=h0_sb, in_=h0T.ap())
                bhhn_sb = p1.tile([128, KC, BL], dt.float32)
                nc.sync.dma_start(out=bhhn_sb, in_=bhhn.ap())
                nc.vector.tensor_copy(out=h_bf[:, 0, :, :], in_=h0_sb[:])

                CH = 16
                gx_chunks = []
                for c in range(T // CH):
                    gxc = gxcp.tile([128, MC, CH * BL], dt.float32, tag="gxc")
                    nc.sync.dma_start(out=gxc,
                                      in_=gxT_d.ap().rearrange("mc p c -> p mc c")
                                      [:, :, c * CH * BL:(c + 1) * CH * BL])
                    gx_chunks.append(gxc)

                i_last_mm = None
                for t in range(T):
                    c, j = divmod(t, CH)
                    h2 = h_bf[:, t % 2, :, :]
                    g_r = psG.tile([128, KC, BL], dt.float32, tag="gr")
                    g_n = psG.tile([128, KC, BL], dt.float32, tag="gn")
                    g_z = psG.tile([128, KC, BL], dt.float32, tag="gz")
                    gxc_t = gx_chunks[c]
                    # r gate W_hh matmuls. First one pinned after the previous
                    # step's last matmul so the scheduler's PE order matches
                    # emission order.
                    for mc in range(KC):
                        for kc in range(KC):
                            i_mm = nc.tensor.matmul(
                                g_r[:, mc, :], w_sb[:, kc, mc, :],
                                h2[:, kc, :],
                                start=(kc == 0), stop=(kc == KC - 1),
                                skip_group_check=True)
                            if mc == 0 and kc == 0 and i_last_mm is not None:
                                tile.add_dep_helper(i_mm.ins, i_last_mm.ins,
                                                    sync=True,
                                                    reason="pin after prev step")
                    r_s = p1t.tile([128, KC, BL], dt.float32, tag="r_s")
                    nc.vector.tensor_add(r_s[:], g_r[:],
                                         gxc_t[:, 0:KC, j * BL:(j + 1) * BL])
                    nc.scalar.activation(out=r_s[:], in_=r_s[:], func=AF.Sigmoid)
                    # n gate W_hh matmuls
                    for mc in range(16, MC):
                        for kc in range(KC):
                            nc.tensor.matmul(g_n[:, mc - 16, :], w_sb[:, kc, mc, :],
                                             h2[:, kc, :],
                                             start=(kc == 0), stop=(kc == KC - 1))
                    gxt = gxc_t[:, 2 * KC:MC, j * BL:(j + 1) * BL]
                    hold = h0_sb[:] if t == 0 else catT[:, 0:KC, t - 1, :]
                    tn = p1t.tile([128, KC, BL], dt.float32, tag="tn")
                    nc.vector.tensor_add(tn[:], g_n[:], bhhn_sb[:])
                    nc.vector.tensor_mul(tn[:], tn[:], r_s[:])
                    nc.vector.tensor_add(tn[:], tn[:], gxt[:])
                    i_tanh = nc.scalar.activation(out=tn[:], in_=tn[:], func=AF.Tanh)
                    tu = p1t.tile([128, KC, BL], dt.float32, tag="tu")
                    nc.vector.tensor_sub(tu[:], hold, tn[:])
                    # z gate W_hh matmuls
                    for mc in range(KC, 16):
                        for kc in range(KC):
                            i_last_mm = nc.tensor.matmul(
                                g_z[:, mc - KC, :], w_sb[:, kc, mc, :],
                                h2[:, kc, :],
                                start=(kc == 0), stop=(kc == KC - 1),
                                skip_group_check=True)
                    z_s = p1t.tile([128, KC, BL], dt.float32, tag="z_s")
                    nc.vector.tensor_add(z_s[:], g_z[:],
                                         gxc_t[:, KC:2 * KC, j * BL:(j + 1) * BL])
                    i_zsig = nc.scalar.activation(out=z_s[:], in_=z_s[:],
                                                  func=AF.Sigmoid)
                    # keep ACT FIFO order [.., tanh, z-sig]
                    tile.add_dep_helper(i_zsig.ins, i_tanh.ins, sync=True,
                                        reason="ACT order: tanh before z-sig")
                    nc.vector.tensor_mul(z_s[:], z_s[:], tu[:])
                    # bf16 h on DVE (same-engine FIFO after the mul)
                    nc.vector.tensor_add(h_bf[:, (t + 1) % 2, :, :], z_s[:], tn[:])
                    # f32 catT on POOL (off critical path)
                    nc.gpsimd.tensor_add(catT[:, 0:KC, t, :], z_s[:], tn[:])

            # ---- P2a: attention ----
            with tc.tile_pool(name="p2a", bufs=1) as p2a, \
                 tc.tile_pool(name="p2at", bufs=2) as p2at, \
                 tc.tile_pool(name="p2t", bufs=4) as p2t, \
                 tc.tile_pool(name="psB", bufs=2, space="PSUM") as psB:
                for b in range(BL):
                    encTb = p2a.tile([128, KC, S], dt.float32r, tag="encT")
                    nc.sync.dma_start(out=encTb, in_=encT_d.ap().bitcast(dt.float32r)
                                      .rearrange("kc p b s -> p kc b s")[:, :, b, :])
                    encNb = p2a.tile([128, 4, H], dt.float32r, tag="encN")
                    nc.sync.dma_start(out=encNb, in_=enc_d.ap().bitcast(dt.float32r)
                                      .rearrange("sc p b h -> p sc b h")[:, :, b, :])
                    attnT = p2at.tile([128, 4, T], dt.float32r, tag="attnT")
                    for tcn in range(T // 128):
                        ps_sc = psB.tile([128, S], dt.float32, tag="sc")
                        for kc in range(KC):
                            nc.tensor.matmul(ps_sc[:],
                                             catT[:, kc, tcn * 128:(tcn + 1) * 128, b],
                                             encTb[:, kc, :],
                                             start=(kc == 0), stop=(kc == KC - 1))
                        negmax = p2t.tile([128, 1], dt.float32, tag="mx")
                        nc.vector.tensor_reduce(negmax[:], ps_sc[:],
                                                axis=mybir.AxisListType.X,
                                                op=ALU.max, negate=True)
                        attn = p2t.tile([128, S], dt.float32r, tag="attn")
                        ssum = p2t.tile([128, 1], dt.float32, tag="ssum")
                        nc.scalar.activation(out=attn[:], in_=ps_sc[:], func=AF.Exp,
                                             bias=negmax[:], scale=1.0,
                                             accum_out=ssum[:])
                        rinv = p2t.tile([128, 1], dt.float32, tag="rinv")
                        nc.vector.reciprocal(rinv[:], ssum[:])
                        nc.vector.tensor_scalar_mul(attn[:], attn[:], rinv[:])
                        for sc in range(4):
                            ps_tr = psB.tile([128, 128], dt.float32r, tag="tr")
                            nc.tensor.transpose(ps_tr[:],
                                                attn[:, sc * 128:(sc + 1) * 128],
                                                ident[:])
                            nc.vector.tensor_copy(
                                out=attnT[:, sc, tcn * 128:(tcn + 1) * 128],
                                in_=ps_tr[:])
                    for hc in range(KC):
                        ps_ctx = psB.tile([128, T], dt.float32, tag="ctx")
                        for sc in range(4):
                            nc.tensor.matmul(ps_ctx[:],
                                             encNb[:, sc, hc * 128:(hc + 1) * 128],
                                             attnT[:, sc, :],
                                             start=(sc == 0), stop=(sc == 3))
                        nc.vector.tensor_copy(out=catT[:, KC + hc, :, b], in_=ps_ctx[:])

            # ---- P2b: concat linear + out linear ----
            with tc.tile_pool(name="p2b", bufs=1) as p2b, \
                 tc.tile_pool(name="wc", bufs=6) as wcp, \
                 tc.tile_pool(name="cT", bufs=2) as cTp, \
                 tc.tile_pool(name="p2o", bufs=4) as p2o, \
                 tc.tile_pool(name="psC", bufs=2, space="PSUM") as psC:
                bc_sb = p2b.tile([128, KC], dt.float32)
                nc.sync.dma_start(out=bc_sb, in_=b_cT.ap())
                wo_sb = p2b.tile([128, KC, D], dt.float32r)
                nc.sync.dma_start(out=wo_sb, in_=w_oT.ap().bitcast(dt.float32r)
                                  .rearrange("(kc p) d -> p kc d", p=128))
                bo_sb = p2b.tile([128, D], dt.float32)
                nc.sync.dma_start(out=bo_sb, in_=b_o_b.ap())
                mask_sb = p2b.tile([128, 4, BL], dt.float32)
                nc.sync.dma_start(out=mask_sb, in_=maskTb.ap())
                wcT_ap = w_cT.ap().bitcast(dt.float32r).rearrange(
                    "(kc p) (mc m) -> p kc mc m", p=128, m=128)
                for b in range(BL):
                    cTb = cTp.tile([128, KC, T], dt.float32r, tag="cT")
                    for mc2 in range(KC):
                        ps_c = psC.tile([128, T], dt.float32, tag="c")
                        for kc2 in range(KC2):
                            wt = wcp.tile([128, 128], dt.float32r, tag="wc")
                            nc.sync.dma_start(out=wt, in_=wcT_ap[:, kc2, mc2, :])
                            nc.tensor.matmul(ps_c[:], wt[:], catT[:, kc2, :, b],
                                             start=(kc2 == 0), stop=(kc2 == KC2 - 1))
                        nc.scalar.activation(out=cTb[:, mc2, :], in_=ps_c[:],
                                             func=AF.Tanh,
                                             bias=bc_sb[:, mc2:mc2 + 1], scale=1.0)
                    for tcn in range(T // 128):
                        ps_o = psC.tile([128, D], dt.float32, tag="o")
                        for hc in range(KC):
                            nc.tensor.matmul(ps_o[:],
                                             cTb[:, hc, tcn * 128:(tcn + 1) * 128],
                                             wo_sb[:, hc, :],
                                             start=(hc == 0), stop=(hc == KC - 1))
                        o_sb = p2o.tile([128, D], dt.float32, tag="o_s")
                        nc.vector.tensor_add(o_sb[:], ps_o[:], bo_sb[:])
                        nc.vector.tensor_scalar_mul(o_sb[:], o_sb[:],
                                                    mask_sb[:, tcn, b:b + 1])
                        nc.sync.dma_start(
                            out=out_l.ap()[b, tcn * 128:(tcn + 1) * 128, :],
                            in_=o_sb[:])

    nc.compile()
    return nc


def _prep_inputs(inputs, core):
    boff = core * BL
    enc = np.ascontiguousarray(inputs["encoder_outputs"][boff:boff + BL])
    tgt = inputs["target_tensor"][boff:boff + BL]
    tl = inputs["target_length"][boff:boff + BL]
    h0 = inputs["h0"][0, boff:boff + BL]
    W_ih, W_hh = inputs["W_ih"], inputs["W_hh"]
    b_g = (inputs["b_ih"] + inputs["b_hh"]).astype(np.float32)
    b_g[2 * H:] = inputs["b_ih"][2 * H:]   # b_hh_n goes inside the r-multiply
    bhhn_np = np.broadcast_to(
        inputs["b_hh"][2 * H:].astype(np.float32)
        .reshape(KC, 128).T[:, :, None], (128, KC, BL)).copy()

    xs = np.concatenate([np.zeros((1, BL, D), np.float32),
                         tgt.transpose(1, 0, 2)[:-1]], 0)
    xT = np.ascontiguousarray(xs.reshape(T * BL, D).T)

    return {
        "w_hhT": np.ascontiguousarray(W_hh.T).astype(ml_dtypes.bfloat16),
        "w_ihTn": np.ascontiguousarray(W_ih.T).astype(np.float32),
        "xT": xT.astype(np.float32),
        "bias_nT": np.ascontiguousarray(
            b_g.reshape(MC, 128).T).astype(np.float32),
        "h0T": np.ascontiguousarray(h0.T.reshape(KC, 128, BL).transpose(1, 0, 2)),
        "encT_d": np.ascontiguousarray(
            enc.transpose(2, 1, 0).reshape(KC, 128, S, BL).transpose(0, 1, 3, 2)),
        "enc_d": np.ascontiguousarray(enc.transpose(1, 0, 2).reshape(4, 128, BL, H)),
        "w_cT": np.ascontiguousarray(inputs["W_c"].T).astype(np.float32),
        "b_cT": np.ascontiguousarray(inputs["b_c"].reshape(KC, 128).T),
        "w_oT": np.ascontiguousarray(inputs["W_o"].T),
        "b_o_b": np.broadcast_to(inputs["b_o"], (128, D)).copy(),
        "maskTb": np.ascontiguousarray(
            (np.arange(T)[:, None] < tl[None, :]).astype(np.float32)
            .reshape(4, 128, BL).transpose(1, 0, 2)),
        "bhhn": bhhn_np,
    }


_NC_CACHE = []
LAST_EXEC_NS = None


def _install_trace_shim():
    """antenv.axon_hooks shim so trace=True works under axon in this container."""
    import types, ctypes, contextlib
    if "antenv.axon_hooks" in sys.modules:
        return
    so_path = "/opt/axon/libaxon_pjrt.so"
    hook = None
    if os.path.exists(so_path):
        lib = ctypes.CDLL(so_path)
        if hasattr(lib, "axon_start_nrt_profile"):
            lib.axon_start_nrt_profile.argtypes = [ctypes.POINTER(ctypes.c_int64),
                                                   ctypes.c_size_t]
            lib.axon_start_nrt_profile.restype = ctypes.c_int64
            lib.axon_stop_nrt_profile.argtypes = [ctypes.c_char_p]
            lib.axon_stop_nrt_profile.restype = ctypes.c_int64

            @contextlib.contextmanager
            def _hook(output_dir, device_ids):
                import jax
                jax.devices()
                if device_ids:
                    ids = (ctypes.c_int64 * len(device_ids))(*device_ids)
                    rc = lib.axon_start_nrt_profile(ids, len(device_ids))
                else:
                    rc = lib.axon_start_nrt_profile(None, 0)
                if rc != 0:
                    raise RuntimeError(f"axon_start_nrt_profile rc={rc}")
                try:
                    yield
                finally:
                    n = lib.axon_stop_nrt_profile(str(output_dir).encode())
                    print(f"profile: {n} file(s) written to {output_dir}",
                          file=sys.stderr)
            hook = _hook
    mod = types.ModuleType("antenv.axon_hooks")
    mod.get_axon_ntff_profile_hook = lambda: hook
    mod.set_axon_ntff_profile_hook = lambda h: None
    sys.modules["antenv.axon_hooks"] = mod
    import concourse.bass_utils as bu
    bu.upload_artifacts = lambda tmpdir: f"local://{tmpdir}"


def kernel(**inputs):
    global LAST_EXEC_NS
    inputs = {k: np.asarray(v) for k, v in inputs.items()}
    if not _NC_CACHE:
        _NC_CACHE.append(_build())
    nc = _NC_CACHE[0]
    in_maps = [_prep_inputs(inputs, core) for core in range(NCORES)]
    kwargs = {}
    if os.environ.get("DEC_TRACE") == "1":
        _install_trace_shim()
        import tempfile
        kwargs = dict(trace=True, tmpdir=tempfile.mkdtemp(prefix="dec_trace_"))
    res = run_bass_kernel_spmd(nc, in_maps, core_ids=list(range(NCORES)), **kwargs)
    LAST_EXEC_NS = res.exec_time_ns
    out = np.concatenate([res.results[c]["out_l"] for c in range(NCORES)], axis=0)
    return out.astype(np.float32)



# revision 10
# speedup vs baseline: 1.1440x; 1.1440x over previous
"""Trainium2 Bass kernel: GRU decoder with Luong attention (B=32, T=S=512, H=1024, D=80).

Strategy (8 NeuronCores, data-parallel over batch, 4 sequences per core):
  P0: gx = W_ih @ x precomputed for the n gate only (fp32r matmuls) -> DRAM.
      The r,z gate gx+bias are folded into the recurrent PSUM accumulation
      via an extra matmul per chunk against the augmented input x~=[x;1]
      (weights [W_ih_rz; b_rz], bf16, zero-padded to 128 partitions — the
      PE FWL weight loader corrupts sub-128-partition bf16 loads).
  P1: sequential GRU in transposed layout (H on partitions, batch on free).
      Per step: 16 h-independent gx matmuls (cover the previous step's gate
      tail), then 192 (LDWEIGHTS+MATMUL) pairs streaming W_hh.T bf16 tiles
      against the bf16 h vector. Gate sigmoids read PSUM directly (bias
      folded via x~), the bf16 h for the next step is written by DVE right
      after the z-multiply (same-engine FIFO), and the f32 h copy (catT,
      attention input) goes to GPSIMD off the critical path. Explicit
      scheduling deps pin the gx matmuls behind the previous step's last
      matmul and keep the ACT queue order [tanh, z-sigmoid] — the Tile
      scheduler's cost model underestimates the matmul stream and otherwise
      misorders both.
  P2: attention for all timesteps at once (scoresT -> softmax -> PE
      transpose -> ctxT), then the concat linear (tanh) and output linear
      as fp32r matmuls.

All per-core inputs are sliced/transposed on the host; the 8 cores run the
same NEFF via run_bass_kernel_spmd with per-core input maps.
"""

import os
import sys

for _p in ("/opt/trn_rl_repo", "/root/.axon_site/_ro/trn_rl_repo"):
    if os.path.isdir(_p) and _p not in sys.path:
        sys.path.insert(0, _p)

import numpy as np
import ml_dtypes

import concourse.bass as bass
import concourse.mybir as mybir
import concourse.tile as tile
from concourse import bacc
from concourse.bass_utils import run_bass_kernel_spmd
from concourse.masks import make_identity

dt = mybir.dt
AF = mybir.ActivationFunctionType
ALU = mybir.AluOpType

H, D, B, S, T = 1024, 80, 32, 512, 512
BL = 4          # batch per core
NCORES = 8
KC = 8          # H / 128
MC = 24         # 3H / 128
KC2 = 16        # 2H / 128
NC8 = 8         # n-gate chunk count


def _build():
    nc = bacc.Bacc("TRN2", target_bir_lowering=False, debug=False,
                   num_devices=NCORES)
    f32r = dt.float32r

    w_hhT = nc.dram_tensor("w_hhT", [H, 3 * H], dt.bfloat16, kind="ExternalInput")
    w_ihTn = nc.dram_tensor("w_ihTn", [D, H], dt.float32, kind="ExternalInput")
    xT = nc.dram_tensor("xT", [D, T * BL], dt.float32, kind="ExternalInput")
    xa = nc.dram_tensor("xa", [128, T * BL], dt.bfloat16, kind="ExternalInput")
    wrz = nc.dram_tensor("wrz", [128, 2 * H], dt.bfloat16, kind="ExternalInput")
    bias_nT = nc.dram_tensor("bias_nT", [128, NC8], dt.float32, kind="ExternalInput")
    h0T = nc.dram_tensor("h0T", [128, KC, BL], dt.float32, kind="ExternalInput")
    encT_d = nc.dram_tensor("encT_d", [KC, 128, BL, S], dt.float32, kind="ExternalInput")
    enc_d = nc.dram_tensor("enc_d", [4, 128, BL, H], dt.float32, kind="ExternalInput")
    w_cT = nc.dram_tensor("w_cT", [2 * H, H], dt.float32, kind="ExternalInput")
    b_cT = nc.dram_tensor("b_cT", [128, KC], dt.float32, kind="ExternalInput")
    w_oT = nc.dram_tensor("w_oT", [H, D], dt.float32, kind="ExternalInput")
    b_o_b = nc.dram_tensor("b_o_b", [128, D], dt.float32, kind="ExternalInput")
    maskTb = nc.dram_tensor("maskTb", [128, 4, BL], dt.float32, kind="ExternalInput")
    bhhn = nc.dram_tensor("bhhn", [128, KC, BL], dt.float32, kind="ExternalInput")

    out_l = nc.dram_tensor("out_l", [BL, T, D], dt.float32, kind="ExternalOutput")
    gxT_d = nc.dram_tensor("gxT_d", [NC8, 128, T * BL], dt.float32)

    with tile.TileContext(nc) as tc:
        with tc.tile_pool(name="persist", bufs=1) as persist:
            catT = persist.tile([128, KC2, T, BL], dt.float32r)
            ident_f = persist.tile([128, 128], dt.float32)
            make_identity(nc, ident_f)
            ident = persist.tile([128, 128], dt.float32r)
            nc.vector.tensor_copy(out=ident[:], in_=ident_f[:])

            # ---- P0: gx precompute (n gate only) ----
            with tc.tile_pool(name="p0", bufs=1) as p0, \
                 tc.tile_pool(name="p0o", bufs=4) as p0o, \
                 tc.tile_pool(name="psA", bufs=2, space="PSUM") as psA:
                biasn_sb = p0.tile([128, NC8], dt.float32)
                nc.sync.dma_start(out=biasn_sb, in_=bias_nT.ap())
                xT_sb = p0.tile([D, T * BL], dt.float32r)
                nc.sync.dma_start(out=xT_sb, in_=xT.ap().bitcast(f32r))
                wihn_sb = p0.tile([D, NC8, 128], dt.float32r)
                nc.sync.dma_start(
                    out=wihn_sb,
                    in_=w_ihTn.ap().bitcast(f32r).rearrange("p (mc m) -> p mc m", m=128))
                for mc in range(NC8):
                    for nt in range(4):
                        ps = psA.tile([128, 512], dt.float32, tag="gx")
                        nc.tensor.matmul(ps[:], wihn_sb[:, mc, :],
                                         xT_sb[:, nt * 512:(nt + 1) * 512],
                                         start=True, stop=True)
                        gxs = p0o.tile([128, 512], dt.float32, tag="gxo")
                        if (mc + nt) % 2 == 0:
                            nc.scalar.activation(out=gxs[:], in_=ps[:], func=AF.Identity,
                                                 bias=biasn_sb[:, mc:mc + 1], scale=1.0)
                        else:
                            nc.vector.tensor_scalar_add(gxs[:], ps[:],
                                                        biasn_sb[:, mc:mc + 1])
                        nc.sync.dma_start(out=gxT_d.ap()[mc, :, nt * 512:(nt + 1) * 512],
                                          in_=gxs[:])

            # ---- P1: sequential GRU ----
            with tc.tile_pool(name="p1w", bufs=1) as p1w, \
                 tc.tile_pool(name="p1", bufs=1) as p1, \
                 tc.tile_pool(name="gxc", bufs=2) as gxcp, \
                 tc.tile_pool(name="p1t", bufs=3) as p1t, \
                 tc.tile_pool(name="psG", bufs=2, space="PSUM") as psG:
                w_sb = p1w.tile([128, KC, MC, 128], dt.bfloat16)
                for kc in range(KC):
                    nc.sync.dma_start(
                        out=w_sb[:, kc, :, :],
                        in_=w_hhT.ap()[kc * 128:(kc + 1) * 128, :]
                            .rearrange("p (mc m) -> p mc m", m=128))
                xa_sb = p1.tile([128, T * BL], dt.bfloat16)
                nc.sync.dma_start(out=xa_sb, in_=xa.ap())
                wrz_sb = p1.tile([128, 16, 128], dt.bfloat16)
                nc.sync.dma_start(
                    out=wrz_sb,
                    in_=wrz.ap().rearrange("p (mc m) -> p mc m", m=128))
                h_bf = p1.tile([128, 2, KC, BL], dt.bfloat16)
                h0_sb = p1.tile([128, KC, BL], dt.float32)
                nc.sync.dma_start(out=h0_sb, in_=h0T.ap())
                bhhn_sb = p1.tile([128, KC, BL], dt.float32)
                nc.sync.dma_start(out=bhhn_sb, in_=bhhn.ap())
                nc.vector.tensor_copy(out=h_bf[:, 0, :, :], in_=h0_sb[:])

                CH = 16
                gx_chunks = []
                for c in range(T // CH):
                    gxc = gxcp.tile([128, NC8, CH * BL], dt.float32, tag="gxc")
                    nc.sync.dma_start(out=gxc,
                                      in_=gxT_d.ap().rearrange("mc p c -> p mc c")
                                      [:, :, c * CH * BL:(c + 1) * CH * BL])
                    gx_chunks.append(gxc)

                i_last_mm = None
                for t in range(T):
                    c, j = divmod(t, CH)
                    h2 = h_bf[:, t % 2, :, :]
                    g_r = psG.tile([128, KC, BL], dt.float32, tag="gr")
                    g_n = psG.tile([128, KC, BL], dt.float32, tag="gn")
                    g_z = psG.tile([128, KC, BL], dt.float32, tag="gz")
                    xa_t = xa_sb[:, t * BL:(t + 1) * BL]
                    # h-independent gx matmuls first: cover the previous tail.
                    # start=True only on the first write per PSUM bank (a
                    # start matmul clears has_written for the whole bank).
                    # Pinned after the previous step's last matmul so the
                    # scheduler's PE order matches emission order.
                    for mc in range(KC):
                        i_gx = nc.tensor.matmul(g_r[:, mc, :],
                                                wrz_sb[:, mc, :], xa_t,
                                                start=(mc == 0), stop=False,
                                                skip_group_check=True)
                        if i_last_mm is not None:
                            tile.add_dep_helper(i_gx.ins, i_last_mm.ins,
                                                sync=True,
                                                reason="pin gx after prev step")
                    for mc in range(KC):
                        i_gx = nc.tensor.matmul(g_z[:, mc, :],
                                                wrz_sb[:, KC + mc, :], xa_t,
                                                start=(mc == 0), stop=False,
                                                skip_group_check=True)
                        if i_last_mm is not None:
                            tile.add_dep_helper(i_gx.ins, i_last_mm.ins,
                                                sync=True,
                                                reason="pin gx after prev step")
                    # r gate W_hh matmuls
                    for mc in range(KC):
                        for kc in range(KC):
                            nc.tensor.matmul(g_r[:, mc, :], w_sb[:, kc, mc, :],
                                             h2[:, kc, :],
                                             start=False, stop=(kc == KC - 1),
                                             skip_group_check=True)
                    r_s = p1t.tile([128, KC, BL], dt.float32, tag="r_s")
                    nc.scalar.activation(out=r_s[:], in_=g_r[:], func=AF.Sigmoid)
                    # n gate W_hh matmuls
                    for mc in range(16, MC):
                        for kc in range(KC):
                            nc.tensor.matmul(g_n[:, mc - 16, :], w_sb[:, kc, mc, :],
                                             h2[:, kc, :],
                                             start=(kc == 0), stop=(kc == KC - 1))
                    gxt = gx_chunks[c][:, :, j * BL:(j + 1) * BL]
                    hold = h0_sb[:] if t == 0 else catT[:, 0:KC, t - 1, :]
                    tn = p1t.tile([128, KC, BL], dt.float32, tag="tn")
                    nc.vector.tensor_add(tn[:], g_n[:], bhhn_sb[:])
                    nc.vector.tensor_mul(tn[:], tn[:], r_s[:])
                    nc.vector.tensor_add(tn[:], tn[:], gxt[:])
                    i_tanh = nc.scalar.activation(out=tn[:], in_=tn[:], func=AF.Tanh)
                    tu = p1t.tile([128, KC, BL], dt.float32, tag="tu")
                    nc.vector.tensor_sub(tu[:], hold, tn[:])
                    # z gate W_hh matmuls
                    for mc in range(KC, 16):
                        for kc in range(KC):
                            i_last_mm = nc.tensor.matmul(
                                g_z[:, mc - KC, :], w_sb[:, kc, mc, :],
                                h2[:, kc, :],
                                start=False, stop=(kc == KC - 1),
                                skip_group_check=True)
                    z_s = p1t.tile([128, KC, BL], dt.float32, tag="z_s")
                    i_zsig = nc.scalar.activation(out=z_s[:], in_=g_z[:],
                                                  func=AF.Sigmoid)
                    # keep ACT FIFO order [.., tanh, z-sig]
                    tile.add_dep_helper(i_zsig.ins, i_tanh.ins, sync=True,
                                        reason="ACT order: tanh before z-sig")
                    nc.vector.tensor_mul(z_s[:], z_s[:], tu[:])
                    # bf16 h on DVE (same-engine FIFO after the mul)
                    nc.vector.tensor_add(h_bf[:, (t + 1) % 2, :, :], z_s[:], tn[:])
                    # f32 catT on POOL (off critical path)
                    nc.gpsimd.tensor_add(catT[:, 0:KC, t, :], z_s[:], tn[:])

            # ---- P2a: attention ----
            with tc.tile_pool(name="p2a", bufs=1) as p2a, \
                 tc.tile_pool(name="p2at", bufs=2) as p2at, \
                 tc.tile_pool(name="p2t", bufs=4) as p2t, \
                 tc.tile_pool(name="psB", bufs=2, space="PSUM") as psB:
                for b in range(BL):
                    encTb = p2a.tile([128, KC, S], dt.float32r, tag="encT")
                    nc.sync.dma_start(out=encTb, in_=encT_d.ap().bitcast(dt.float32r)
                                      .rearrange("kc p b s -> p kc b s")[:, :, b, :])
                    encNb = p2a.tile([128, 4, H], dt.float32r, tag="encN")
                    nc.sync.dma_start(out=encNb, in_=enc_d.ap().bitcast(dt.float32r)
                                      .rearrange("sc p b h -> p sc b h")[:, :, b, :])
                    attnT = p2at.tile([128, 4, T], dt.float32r, tag="attnT")
                    for tcn in range(T // 128):
                        ps_sc = psB.tile([128, S], dt.float32, tag="sc")
                        for kc in range(KC):
                            nc.tensor.matmul(ps_sc[:],
                                             catT[:, kc, tcn * 128:(tcn + 1) * 128, b],
                                             encTb[:, kc, :],
                                             start=(kc == 0), stop=(kc == KC - 1))
                        negmax = p2t.tile([128, 1], dt.float32, tag="mx")
                        nc.vector.tensor_reduce(negmax[:], ps_sc[:],
                                                axis=mybir.AxisListType.X,
                                                op=ALU.max, negate=True)
                        attn = p2t.tile([128, S], dt.float32r, tag="attn")
                        ssum = p2t.tile([128, 1], dt.float32, tag="ssum")
                        nc.scalar.activation(out=attn[:], in_=ps_sc[:], func=AF.Exp,
                                             bias=negmax[:], scale=1.0,
                                             accum_out=ssum[:])
                        rinv = p2t.tile([128, 1], dt.float32, tag="rinv")
                        nc.vector.reciprocal(rinv[:], ssum[:])
                        nc.vector.tensor_scalar_mul(attn[:], attn[:], rinv[:])
                        for sc in range(4):
                            ps_tr = psB.tile([128, 128], dt.float32r, tag="tr")
                            nc.tensor.transpose(ps_tr[:],
                                                attn[:, sc * 128:(sc + 1) * 128],
                                                ident[:])
                            nc.vector.tensor_copy(
                                out=attnT[:, sc, tcn * 128:(tcn + 1) * 128],
                                in_=ps_tr[:])
                    for hc in range(KC):
                        ps_ctx = psB.tile([128, T], dt.float32, tag="ctx")
                        for sc in range(4):
                            nc.tensor.matmul(ps_ctx[:],
                                             encNb[:, sc, hc * 128:(hc + 1) * 128],
                                             attnT[:, sc, :],
                                             start=(sc == 0), stop=(sc == 3))
                        nc.vector.tensor_copy(out=catT[:, KC + hc, :, b], in_=ps_ctx[:])

            # ---- P2b: concat linear + out linear ----
            with tc.tile_pool(name="p2b", bufs=1) as p2b, \
                 tc.tile_pool(name="wc", bufs=6) as wcp, \
                 tc.tile_pool(name="cT", bufs=2) as cTp, \
                 tc.tile_pool(name="p2o", bufs=4) as p2o, \
                 tc.tile_pool(name="psC", bufs=2, space="PSUM") as psC:
                bc_sb = p2b.tile([128, KC], dt.float32)
                nc.sync.dma_start(out=bc_sb, in_=b_cT.ap())
                wo_sb = p2b.tile([128, KC, D], dt.float32r)
                nc.sync.dma_start(out=wo_sb, in_=w_oT.ap().bitcast(dt.float32r)
                                  .rearrange("(kc p) d -> p kc d", p=128))
                bo_sb = p2b.tile([128, D], dt.float32)
                nc.sync.dma_start(out=bo_sb, in_=b_o_b.ap())
                mask_sb = p2b.tile([128, 4, BL], dt.float32)
                nc.sync.dma_start(out=mask_sb, in_=maskTb.ap())
                wcT_ap = w_cT.ap().bitcast(dt.float32r).rearrange(
                    "(kc p) (mc m) -> p kc mc m", p=128, m=128)
                for b in range(BL):
                    cTb = cTp.tile([128, KC, T], dt.float32r, tag="cT")
                    for mc2 in range(KC):
                        ps_c = psC.tile([128, T], dt.float32, tag="c")
                        for kc2 in range(KC2):
                            wt = wcp.tile([128, 128], dt.float32r, tag="wc")
                            nc.sync.dma_start(out=wt, in_=wcT_ap[:, kc2, mc2, :])
                            nc.tensor.matmul(ps_c[:], wt[:], catT[:, kc2, :, b],
                                             start=(kc2 == 0), stop=(kc2 == KC2 - 1))
                        nc.scalar.activation(out=cTb[:, mc2, :], in_=ps_c[:],
                                             func=AF.Tanh,
                                             bias=bc_sb[:, mc2:mc2 + 1], scale=1.0)
                    for tcn in range(T // 128):
                        ps_o = psC.tile([128, D], dt.float32, tag="o")
                        for hc in range(KC):
                            nc.tensor.matmul(ps_o[:],
                                             cTb[:, hc, tcn * 128:(tcn + 1) * 128],
                                             wo_sb[:, hc, :],
                                             start=(hc == 0), stop=(hc == KC - 1))
                        o_sb = p2o.tile([128, D], dt.float32, tag="o_s")
                        nc.vector.tensor_add(o_sb[:], ps_o[:], bo_sb[:])
                        nc.vector.tensor_scalar_mul(o_sb[:], o_sb[:],
                                                    mask_sb[:, tcn, b:b + 1])
                        nc.sync.dma_start(
                            out=out_l.ap()[b, tcn * 128:(tcn + 1) * 128, :],
                            in_=o_sb[:])

    nc.compile()
    return nc


def _prep_inputs(inputs, core):
    boff = core * BL
    enc = np.ascontiguousarray(inputs["encoder_outputs"][boff:boff + BL])
    tgt = inputs["target_tensor"][boff:boff + BL]
    tl = inputs["target_length"][boff:boff + BL]
    h0 = inputs["h0"][0, boff:boff + BL]
    W_ih, W_hh = inputs["W_ih"], inputs["W_hh"]
    b_g = (inputs["b_ih"] + inputs["b_hh"]).astype(np.float32)
    b_g[2 * H:] = inputs["b_ih"][2 * H:]   # b_hh_n goes inside the r-multiply
    bhhn_np = np.broadcast_to(
        inputs["b_hh"][2 * H:].astype(np.float32)
        .reshape(KC, 128).T[:, :, None], (128, KC, BL)).copy()

    xs = np.concatenate([np.zeros((1, BL, D), np.float32),
                         tgt.transpose(1, 0, 2)[:-1]], 0)
    xT = np.ascontiguousarray(xs.reshape(T * BL, D).T)
    xa_np = np.zeros((128, T * BL), np.float32)
    xa_np[:D] = xT
    xa_np[D] = 1.0
    wrz_np = np.zeros((128, 2 * H), np.float32)
    wrz_np[:D] = W_ih.T[:, :2 * H]
    wrz_np[D] = b_g[:2 * H]

    return {
        "w_hhT": np.ascontiguousarray(W_hh.T).astype(ml_dtypes.bfloat16),
        "w_ihTn": np.ascontiguousarray(W_ih.T[:, 2 * H:]).astype(np.float32),
        "xT": xT.astype(np.float32),
        "xa": xa_np.astype(ml_dtypes.bfloat16),
        "wrz": wrz_np.astype(ml_dtypes.bfloat16),
        "bias_nT": np.ascontiguousarray(
            b_g[2 * H:].reshape(NC8, 128).T).astype(np.float32),
        "h0T": np.ascontiguousarray(h0.T.reshape(KC, 128, BL).transpose(1, 0, 2)),
        "encT_d": np.ascontiguousarray(
            enc.transpose(2, 1, 0).reshape(KC, 128, S, BL).transpose(0, 1, 3, 2)),
        "enc_d": np.ascontiguousarray(enc.transpose(1, 0, 2).reshape(4, 128, BL, H)),
        "w_cT": np.ascontiguousarray(inputs["W_c"].T).astype(np.float32),
        "b_cT": np.ascontiguousarray(inputs["b_c"].reshape(KC, 128).T),
        "w_oT": np.ascontiguousarray(inputs["W_o"].T),
        "b_o_b": np.broadcast_to(inputs["b_o"], (128, D)).copy(),
        "maskTb": np.ascontiguousarray(
            (np.arange(T)[:, None] < tl[None, :]).astype(np.float32)
            .reshape(4, 128, BL).transpose(1, 0, 2)),
        "bhhn": bhhn_np,
    }


_NC_CACHE = []
LAST_EXEC_NS = None


def _install_trace_shim():
    """antenv.axon_hooks shim so trace=True works under axon in this container."""
    import types, ctypes, contextlib
    if "antenv.axon_hooks" in sys.modules:
        return
    so_path = "/opt/axon/libaxon_pjrt.so"
    hook = None
    if os.path.exists(so_path):
        lib = ctypes.CDLL(so_path)
        if hasattr(lib, "axon_start_nrt_profile"):
            lib.axon_start_nrt_profile.argtypes = [ctypes.POINTER(ctypes.c_int64),
                                                   ctypes.c_size_t]
            lib.axon_start_nrt_profile.restype = ctypes.c_int64
            lib.axon_stop_nrt_profile.argtypes = [ctypes.c_char_p]
            lib.axon_stop_nrt_profile.restype = ctypes.c_int64

            @contextlib.contextmanager
            def _hook(output_dir, device_ids):
                import jax
                jax.devices()
                if device_ids:
                    ids = (ctypes.c_int64 * len(device_ids))(*device_ids)
                    rc = lib.axon_start_nrt_profile(ids, len(device_ids))
                else:
                    rc = lib.axon_start_nrt_profile(None, 0)
                if rc != 0:
                    raise RuntimeError(f"axon_start_nrt_profile rc={rc}")
                try:
                    yield
                finally:
                    n = lib.axon_stop_nrt_profile(str(output_dir).encode())
                    print(f"profile: {n} file(s) written to {output_dir}",
                          file=sys.stderr)
            hook = _hook
    mod = types.ModuleType("antenv.axon_hooks")
    mod.get_axon_ntff_profile_hook = lambda: hook
    mod.set_axon_ntff_profile_hook = lambda h: None
    sys.modules["antenv.axon_hooks"] = mod
    import concourse.bass_utils as bu
    bu.upload_artifacts = lambda tmpdir: f"local://{tmpdir}"


def kernel(**inputs):
    global LAST_EXEC_NS
    inputs = {k: np.asarray(v) for k, v in inputs.items()}
    if not _NC_CACHE:
        _NC_CACHE.append(_build())
    nc = _NC_CACHE[0]
    in_maps = [_prep_inputs(inputs, core) for core in range(NCORES)]
    kwargs = {}
    if os.environ.get("DEC_TRACE") == "1":
        _install_trace_shim()
        import tempfile
        kwargs = dict(trace=True, tmpdir=tempfile.mkdtemp(prefix="dec_trace_"))
    res = run_bass_kernel_spmd(nc, in_maps, core_ids=list(range(NCORES)), **kwargs)
    LAST_EXEC_NS = res.exec_time_ns
    out = np.concatenate([res.results[c]["out_l"] for c in range(NCORES)], axis=0)
    return out.astype(np.float32)



# revision 12
# speedup vs baseline: 1.1755x; 1.0276x over previous
"""Trainium2 Bass kernel: GRU decoder with Luong attention (B=32, T=S=512, H=1024, D=80).

Strategy (8 NeuronCores, data-parallel over batch, 4 sequences per core):
  P0: gx = W_ih @ x precomputed for the n gate only (fp32r matmuls) -> DRAM.
      The r,z gate gx+bias are folded into the recurrent PSUM accumulation
      via an extra matmul per chunk against the augmented input x~=[x;1]
      (weights [W_ih_rz; b_rz], bf16, zero-padded to 128 partitions — the
      PE FWL weight loader corrupts sub-128-partition bf16 loads).
  P1: sequential GRU in transposed layout (H on partitions, batch on free).
      Per step: 16 h-independent gx matmuls (cover the previous step's gate
      tail), then 192 (LDWEIGHTS+MATMUL) pairs streaming W_hh.T bf16 tiles
      against the bf16 h vector. Gate sigmoids read PSUM directly (bias
      folded via x~), the bf16 h for the next step is written by DVE right
      after the z-multiply (same-engine FIFO), and the f32 h copy (catT,
      attention input) goes to GPSIMD off the critical path. Explicit
      scheduling deps pin the gx matmuls behind the previous step's last
      matmul and keep the ACT queue order [tanh, z-sigmoid] — the Tile
      scheduler's cost model underestimates the matmul stream and otherwise
      misorders both.
  P2: attention for all timesteps at once (scoresT -> softmax -> PE
      transpose -> ctxT), then the concat linear (tanh) and output linear
      as fp32r matmuls.

All per-core inputs are sliced/transposed on the host; the 8 cores run the
same NEFF via run_bass_kernel_spmd with per-core input maps.
"""

import os
import sys

for _p in ("/opt/trn_rl_repo", "/root/.axon_site/_ro/trn_rl_repo"):
    if os.path.isdir(_p) and _p not in sys.path:
        sys.path.insert(0, _p)

import numpy as np
import ml_dtypes

import concourse.bass as bass
import concourse.mybir as mybir
import concourse.tile as tile
from concourse import bacc
from concourse.bass_utils import run_bass_kernel_spmd
from concourse.masks import make_identity

dt = mybir.dt
AF = mybir.ActivationFunctionType
ALU = mybir.AluOpType

H, D, B, S, T = 1024, 80, 32, 512, 512
BL = 4          # batch per core
NCORES = 8
KC = 8          # H / 128
MC = 24         # 3H / 128
KC2 = 16        # 2H / 128
NC8 = 8         # n-gate chunk count


def _build():
    nc = bacc.Bacc("TRN2", target_bir_lowering=False, debug=False,
                   num_devices=NCORES)
    f32r = dt.float32r

    w_hhT = nc.dram_tensor("w_hhT", [H, 3 * H], dt.bfloat16, kind="ExternalInput")
    w_ihTn = nc.dram_tensor("w_ihTn", [D, H], dt.float32, kind="ExternalInput")
    xT = nc.dram_tensor("xT", [D, T * BL], dt.float32, kind="ExternalInput")
    xa = nc.dram_tensor("xa", [128, T * BL], dt.bfloat16, kind="ExternalInput")
    wrz = nc.dram_tensor("wrz", [128, 2 * H], dt.bfloat16, kind="ExternalInput")
    bias_nT = nc.dram_tensor("bias_nT", [128, NC8], dt.float32, kind="ExternalInput")
    h0T = nc.dram_tensor("h0T", [128, KC, BL], dt.float32, kind="ExternalInput")
    encT_d = nc.dram_tensor("encT_d", [KC, 128, BL, S], dt.float32, kind="ExternalInput")
    enc_d = nc.dram_tensor("enc_d", [4, 128, BL, H], dt.float32, kind="ExternalInput")
    w_cT = nc.dram_tensor("w_cT", [2 * H, H], dt.float32, kind="ExternalInput")
    b_cT = nc.dram_tensor("b_cT", [128, KC], dt.float32, kind="ExternalInput")
    w_oT = nc.dram_tensor("w_oT", [H, D], dt.float32, kind="ExternalInput")
    b_o_b = nc.dram_tensor("b_o_b", [128, D], dt.float32, kind="ExternalInput")
    maskTb = nc.dram_tensor("maskTb", [128, 4, BL], dt.float32, kind="ExternalInput")
    bhhn = nc.dram_tensor("bhhn", [128, KC, BL], dt.float32, kind="ExternalInput")

    out_l = nc.dram_tensor("out_l", [BL, T, D], dt.float32, kind="ExternalOutput")
    gxT_d = nc.dram_tensor("gxT_d", [NC8, 128, T * BL], dt.float32)

    with tile.TileContext(nc) as tc:
        with tc.tile_pool(name="persist", bufs=1) as persist:
            catT = persist.tile([128, KC2, T, BL], dt.float32r)
            ident_f = persist.tile([128, 128], dt.float32)
            make_identity(nc, ident_f)
            ident = persist.tile([128, 128], dt.float32r)
            nc.vector.tensor_copy(out=ident[:], in_=ident_f[:])

            # ---- P0: gx precompute (n gate only) ----
            with tc.tile_pool(name="p0", bufs=1) as p0, \
                 tc.tile_pool(name="p0o", bufs=4) as p0o, \
                 tc.tile_pool(name="psA", bufs=2, space="PSUM") as psA:
                biasn_sb = p0.tile([128, NC8], dt.float32)
                nc.sync.dma_start(out=biasn_sb, in_=bias_nT.ap())
                xT_sb = p0.tile([D, T * BL], dt.float32r)
                nc.sync.dma_start(out=xT_sb, in_=xT.ap().bitcast(f32r))
                wihn_sb = p0.tile([D, NC8, 128], dt.float32r)
                nc.sync.dma_start(
                    out=wihn_sb,
                    in_=w_ihTn.ap().bitcast(f32r).rearrange("p (mc m) -> p mc m", m=128))
                for mc in range(NC8):
                    for nt in range(4):
                        ps = psA.tile([128, 512], dt.float32, tag="gx")
                        nc.tensor.matmul(ps[:], wihn_sb[:, mc, :],
                                         xT_sb[:, nt * 512:(nt + 1) * 512],
                                         start=True, stop=True)
                        gxs = p0o.tile([128, 512], dt.float32, tag="gxo")
                        if (mc + nt) % 2 == 0:
                            nc.scalar.activation(out=gxs[:], in_=ps[:], func=AF.Identity,
                                                 bias=biasn_sb[:, mc:mc + 1], scale=1.0)
                        else:
                            nc.vector.tensor_scalar_add(gxs[:], ps[:],
                                                        biasn_sb[:, mc:mc + 1])
                        nc.sync.dma_start(out=gxT_d.ap()[mc, :, nt * 512:(nt + 1) * 512],
                                          in_=gxs[:])

            # ---- P1: sequential GRU ----
            with tc.tile_pool(name="p1w", bufs=1) as p1w, \
                 tc.tile_pool(name="p1", bufs=1) as p1, \
                 tc.tile_pool(name="gxc", bufs=2) as gxcp, \
                 tc.tile_pool(name="p1t", bufs=3) as p1t, \
                 tc.tile_pool(name="psG", bufs=2, space="PSUM") as psG:
                w_sb = p1w.tile([128, KC, MC, 128], dt.bfloat16)
                for kc in range(KC):
                    nc.sync.dma_start(
                        out=w_sb[:, kc, :, :],
                        in_=w_hhT.ap()[kc * 128:(kc + 1) * 128, :]
                            .rearrange("p (mc m) -> p mc m", m=128))
                xa_sb = p1.tile([128, T * BL], dt.bfloat16)
                nc.sync.dma_start(out=xa_sb, in_=xa.ap())
                wrz_sb = p1.tile([128, 16, 128], dt.bfloat16)
                nc.sync.dma_start(
                    out=wrz_sb,
                    in_=wrz.ap().rearrange("p (mc m) -> p mc m", m=128))
                h_bf = p1.tile([128, 2, KC, BL], dt.bfloat16)
                h0_sb = p1.tile([128, KC, BL], dt.float32)
                nc.sync.dma_start(out=h0_sb, in_=h0T.ap())
                bhhn_sb = p1.tile([128, KC, BL], dt.float32)
                nc.sync.dma_start(out=bhhn_sb, in_=bhhn.ap())
                nc.vector.tensor_copy(out=h_bf[:, 0, :, :], in_=h0_sb[:])

                CH = 16
                gx_chunks = []
                for c in range(T // CH):
                    gxc = gxcp.tile([128, NC8, CH * BL], dt.float32, tag="gxc")
                    nc.sync.dma_start(out=gxc,
                                      in_=gxT_d.ap().rearrange("mc p c -> p mc c")
                                      [:, :, c * CH * BL:(c + 1) * CH * BL])
                    gx_chunks.append(gxc)

                i_last_mm = None
                for t in range(T):
                    c, j = divmod(t, CH)
                    h2 = h_bf[:, t % 2, :, :]
                    g_r = psG.tile([128, KC, BL], dt.float32, tag="gr")
                    g_n = psG.tile([128, KC, BL], dt.float32, tag="gn")
                    g_z = psG.tile([128, KC, BL], dt.float32, tag="gz")
                    xa_t = xa_sb[:, t * BL:(t + 1) * BL]
                    # h-independent gx matmuls first: cover the previous tail.
                    # start=True only on the first write per PSUM bank (a
                    # start matmul clears has_written for the whole bank).
                    # Pinned after the previous step's last matmul so the
                    # scheduler's PE order matches emission order.
                    for mc in range(KC):
                        i_gx = nc.tensor.matmul(g_r[:, mc, :],
                                                wrz_sb[:, mc, :], xa_t,
                                                start=(mc == 0), stop=False,
                                                skip_group_check=True)
                        if i_last_mm is not None:
                            tile.add_dep_helper(i_gx.ins, i_last_mm.ins,
                                                sync=True,
                                                reason="pin gx after prev step")
                    for mc in range(KC):
                        i_gx = nc.tensor.matmul(g_z[:, mc, :],
                                                wrz_sb[:, KC + mc, :], xa_t,
                                                start=(mc == 0), stop=False,
                                                skip_group_check=True)
                        if i_last_mm is not None:
                            tile.add_dep_helper(i_gx.ins, i_last_mm.ins,
                                                sync=True,
                                                reason="pin gx after prev step")
                    # r gate W_hh matmuls
                    for mc in range(KC):
                        for kc in range(KC):
                            nc.tensor.matmul(g_r[:, mc, :], w_sb[:, kc, mc, :],
                                             h2[:, kc, :],
                                             start=False, stop=(kc == KC - 1),
                                             skip_group_check=True)
                    r_s = p1t.tile([128, KC, BL], dt.float32, tag="r_s")
                    nc.scalar.activation(out=r_s[:], in_=g_r[:], func=AF.Sigmoid)
                    # n gate W_hh matmuls
                    for mc in range(16, MC):
                        for kc in range(KC):
                            nc.tensor.matmul(g_n[:, mc - 16, :], w_sb[:, kc, mc, :],
                                             h2[:, kc, :],
                                             start=(kc == 0), stop=(kc == KC - 1))
                    gxt = gx_chunks[c][:, :, j * BL:(j + 1) * BL]
                    hold = h0_sb[:] if t == 0 else catT[:, 0:KC, t - 1, :]
                    tn = p1t.tile([128, KC, BL], dt.float32, tag="tn")
                    nc.vector.tensor_add(tn[:], g_n[:], bhhn_sb[:])
                    nc.vector.tensor_mul(tn[:], tn[:], r_s[:])
                    nc.vector.tensor_add(tn[:], tn[:], gxt[:])
                    i_tanh = nc.scalar.activation(out=tn[:], in_=tn[:], func=AF.Tanh)
                    tu = p1t.tile([128, KC, BL], dt.float32, tag="tu")
                    nc.vector.tensor_sub(tu[:], hold, tn[:])
                    # z gate W_hh matmuls
                    for mc in range(KC, 16):
                        for kc in range(KC):
                            i_last_mm = nc.tensor.matmul(
                                g_z[:, mc - KC, :], w_sb[:, kc, mc, :],
                                h2[:, kc, :],
                                start=False, stop=(kc == KC - 1),
                                skip_group_check=True)
                    z_s = p1t.tile([128, KC, BL], dt.float32, tag="z_s")
                    i_zsig = nc.scalar.activation(out=z_s[:], in_=g_z[:],
                                                  func=AF.Sigmoid)
                    # keep ACT FIFO order [.., tanh, z-sig]
                    tile.add_dep_helper(i_zsig.ins, i_tanh.ins, sync=True,
                                        reason="ACT order: tanh before z-sig")
                    nc.vector.tensor_mul(z_s[:], z_s[:], tu[:])
                    # bf16 h on DVE (same-engine FIFO after the mul)
                    nc.vector.tensor_add(h_bf[:, (t + 1) % 2, :, :], z_s[:], tn[:])
                    # f32 catT on POOL (off critical path)
                    nc.gpsimd.tensor_add(catT[:, 0:KC, t, :], z_s[:], tn[:])

            # ---- P2a: attention ----
            with tc.tile_pool(name="p2a", bufs=1) as p2a, \
                 tc.tile_pool(name="p2at", bufs=2) as p2at, \
                 tc.tile_pool(name="p2t", bufs=4) as p2t, \
                 tc.tile_pool(name="psB", bufs=2, space="PSUM") as psB:
                for b in range(BL):
                    encTb = p2a.tile([128, KC, S], dt.float32r, tag="encT")
                    nc.sync.dma_start(out=encTb, in_=encT_d.ap().bitcast(dt.float32r)
                                      .rearrange("kc p b s -> p kc b s")[:, :, b, :])
                    encNb = p2a.tile([128, 4, H], dt.float32r, tag="encN")
                    nc.sync.dma_start(out=encNb, in_=enc_d.ap().bitcast(dt.float32r)
                                      .rearrange("sc p b h -> p sc b h")[:, :, b, :])
                    attnT = p2at.tile([128, 4, T], dt.float32r, tag="attnT")
                    for tcn in range(T // 128):
                        ps_sc = psB.tile([128, S], dt.float32, tag="sc")
                        for kc in range(KC):
                            nc.tensor.matmul(ps_sc[:],
                                             catT[:, kc, tcn * 128:(tcn + 1) * 128, b],
                                             encTb[:, kc, :],
                                             start=(kc == 0), stop=(kc == KC - 1))
                        negmax = p2t.tile([128, 1], dt.float32, tag="mx")
                        nc.vector.tensor_reduce(negmax[:], ps_sc[:],
                                                axis=mybir.AxisListType.X,
                                                op=ALU.max, negate=True)
                        attn = p2t.tile([128, S], dt.float32r, tag="attn")
                        ssum = p2t.tile([128, 1], dt.float32, tag="ssum")
                        nc.scalar.activation(out=attn[:], in_=ps_sc[:], func=AF.Exp,
                                             bias=negmax[:], scale=1.0,
                                             accum_out=ssum[:])
                        rinv = p2t.tile([128, 1], dt.float32, tag="rinv")
                        nc.vector.reciprocal(rinv[:], ssum[:])
                        nc.vector.tensor_scalar_mul(attn[:], attn[:], rinv[:])
                        for sc in range(4):
                            ps_tr = psB.tile([128, 128], dt.float32r, tag="tr")
                            nc.tensor.transpose(ps_tr[:],
                                                attn[:, sc * 128:(sc + 1) * 128],
                                                ident[:])
                            nc.vector.tensor_copy(
                                out=attnT[:, sc, tcn * 128:(tcn + 1) * 128],
                                in_=ps_tr[:])
                    for hc in range(KC):
                        ps_ctx = psB.tile([128, T], dt.float32, tag="ctx")
                        for sc in range(4):
                            nc.tensor.matmul(ps_ctx[:],
                                             encNb[:, sc, hc * 128:(hc + 1) * 128],
                                             attnT[:, sc, :],
                                             start=(sc == 0), stop=(sc == 3))
                        nc.vector.tensor_copy(out=catT[:, KC + hc, :, b], in_=ps_ctx[:])

            # ---- P2b: concat linear + out linear ----
            with tc.tile_pool(name="p2b", bufs=1) as p2b, \
                 tc.tile_pool(name="wc", bufs=6) as wcp, \
                 tc.tile_pool(name="cT", bufs=4) as cTp, \
                 tc.tile_pool(name="p2o", bufs=4) as p2o, \
                 tc.tile_pool(name="psC", bufs=1, space="PSUM") as psC, \
                 tc.tile_pool(name="psC2", bufs=2, space="PSUM") as psC2:
                bc_sb = p2b.tile([128, KC], dt.float32)
                nc.sync.dma_start(out=bc_sb, in_=b_cT.ap())
                wo_sb = p2b.tile([128, KC, D], dt.float32r)
                nc.sync.dma_start(out=wo_sb, in_=w_oT.ap().bitcast(dt.float32r)
                                  .rearrange("(kc p) d -> p kc d", p=128))
                bo_sb = p2b.tile([128, D], dt.float32)
                nc.sync.dma_start(out=bo_sb, in_=b_o_b.ap())
                mask_sb = p2b.tile([128, 4, BL], dt.float32)
                nc.sync.dma_start(out=mask_sb, in_=maskTb.ap())
                wcT_ap = w_cT.ap().bitcast(dt.float32r).rearrange(
                    "(kc p) (mc m) -> p kc mc m", p=128, m=128)
                # concat linear: one W_c tile load serves all 4 batches
                # (4x less DMA, denser PE stream than per-batch streaming)
                cTbs = [cTp.tile([128, KC, T], dt.float32r, tag="cT",
                                 name=f"cTb{b}") for b in range(BL)]
                for mc2 in range(KC):
                    pss = [psC.tile([128, T], dt.float32, tag=f"c{b}",
                                    name=f"psc{b}") for b in range(BL)]
                    for kc2 in range(KC2):
                        wt = wcp.tile([128, 128], dt.float32r, tag="wc")
                        nc.sync.dma_start(out=wt, in_=wcT_ap[:, kc2, mc2, :])
                        for b in range(BL):
                            nc.tensor.matmul(pss[b][:], wt[:], catT[:, kc2, :, b],
                                             start=(kc2 == 0),
                                             stop=(kc2 == KC2 - 1))
                    for b in range(BL):
                        nc.scalar.activation(out=cTbs[b][:, mc2, :], in_=pss[b][:],
                                             func=AF.Tanh,
                                             bias=bc_sb[:, mc2:mc2 + 1], scale=1.0)
                for b in range(BL):
                    cTb = cTbs[b]
                    for tcn in range(T // 128):
                        ps_o = psC2.tile([128, D], dt.float32, tag="o")
                        for hc in range(KC):
                            nc.tensor.matmul(ps_o[:],
                                             cTb[:, hc, tcn * 128:(tcn + 1) * 128],
                                             wo_sb[:, hc, :],
                                             start=(hc == 0), stop=(hc == KC - 1))
                        o_sb = p2o.tile([128, D], dt.float32, tag="o_s")
                        nc.vector.tensor_add(o_sb[:], ps_o[:], bo_sb[:])
                        nc.vector.tensor_scalar_mul(o_sb[:], o_sb[:],
                                                    mask_sb[:, tcn, b:b + 1])
                        nc.sync.dma_start(
                            out=out_l.ap()[b, tcn * 128:(tcn + 1) * 128, :],
                            in_=o_sb[:])

    nc.compile()
    return nc


def _prep_inputs(inputs, core):
    boff = core * BL
    enc = np.ascontiguousarray(inputs["encoder_outputs"][boff:boff + BL])
    tgt = inputs["target_tensor"][boff:boff + BL]
    tl = inputs["target_length"][boff:boff + BL]
    h0 = inputs["h0"][0, boff:boff + BL]
    W_ih, W_hh = inputs["W_ih"], inputs["W_hh"]
    b_g = (inputs["b_ih"] + inputs["b_hh"]).astype(np.float32)
    b_g[2 * H:] = inputs["b_ih"][2 * H:]   # b_hh_n goes inside the r-multiply
    bhhn_np = np.broadcast_to(
        inputs["b_hh"][2 * H:].astype(np.float32)
        .reshape(KC, 128).T[:, :, None], (128, KC, BL)).copy()

    xs = np.concatenate([np.zeros((1, BL, D), np.float32),
                         tgt.transpose(1, 0, 2)[:-1]], 0)
    xT = np.ascontiguousarray(xs.reshape(T * BL, D).T)
    xa_np = np.zeros((128, T * BL), np.float32)
    xa_np[:D] = xT
    xa_np[D] = 1.0
    wrz_np = np.zeros((128, 2 * H), np.float32)
    wrz_np[:D] = W_ih.T[:, :2 * H]
    wrz_np[D] = b_g[:2 * H]

    return {
        "w_hhT": np.ascontiguousarray(W_hh.T).astype(ml_dtypes.bfloat16),
        "w_ihTn": np.ascontiguousarray(W_ih.T[:, 2 * H:]).astype(np.float32),
        "xT": xT.astype(np.float32),
        "xa": xa_np.astype(ml_dtypes.bfloat16),
        "wrz": wrz_np.astype(ml_dtypes.bfloat16),
        "bias_nT": np.ascontiguousarray(
            b_g[2 * H:].reshape(NC8, 128).T).astype(np.float32),
        "h0T": np.ascontiguousarray(h0.T.reshape(KC, 128, BL).transpose(1, 0, 2)),
        "encT_d": np.ascontiguousarray(
            enc.transpose(2, 1, 0).reshape(KC, 128, S, BL).transpose(0, 1, 3, 2)),
        "enc_d": np.ascontiguousarray(enc.transpose(1, 0, 2).reshape(4, 128, BL, H)),
        "w_cT": np.ascontiguousarray(inputs["W_c"].T).astype(np.float32),
        "b_cT": np.ascontiguousarray(inputs["b_c"].reshape(KC, 128).T),
        "w_oT": np.ascontiguousarray(inputs["W_o"].T),
        "b_o_b": np.broadcast_to(inputs["b_o"], (128, D)).copy(),
        "maskTb": np.ascontiguousarray(
            (np.arange(T)[:, None] < tl[None, :]).astype(np.float32)
            .reshape(4, 128, BL).transpose(1, 0, 2)),
        "bhhn": bhhn_np,
    }


_NC_CACHE = []
LAST_EXEC_NS = None


def _install_trace_shim():
    """antenv.axon_hooks shim so trace=True works under axon in this container."""
    import types, ctypes, contextlib
    if "antenv.axon_hooks" in sys.modules:
        return
    so_path = "/opt/axon/libaxon_pjrt.so"
    hook = None
    if os.path.exists(so_path):
        lib = ctypes.CDLL(so_path)
        if hasattr(lib, "axon_start_nrt_profile"):
            lib.axon_start_nrt_profile.argtypes = [ctypes.POINTER(ctypes.c_int64),
                                                   ctypes.c_size_t]
            lib.axon_start_nrt_profile.restype = ctypes.c_int64
            lib.axon_stop_nrt_profile.argtypes = [ctypes.c_char_p]
            lib.axon_stop_nrt_profile.restype = ctypes.c_int64

            @contextlib.contextmanager
            def _hook(output_dir, device_ids):
                import jax
                jax.devices()
                if device_ids:
                    ids = (ctypes.c_int64 * len(device_ids))(*device_ids)
                    rc = lib.axon_start_nrt_profile(ids, len(device_ids))
                else:
                    rc = lib.axon_start_nrt_profile(None, 0)
                if rc != 0:
                    raise RuntimeError(f"axon_start_nrt_profile rc={rc}")
                try:
                    yield
                finally:
                    n = lib.axon_stop_nrt_profile(str(output_dir).encode())
                    print(f"profile: {n} file(s) written to {output_dir}",
                          file=sys.stderr)
            hook = _hook
    mod = types.ModuleType("antenv.axon_hooks")
    mod.get_axon_ntff_profile_hook = lambda: hook
    mod.set_axon_ntff_profile_hook = lambda h: None
    sys.modules["antenv.axon_hooks"] = mod
    import concourse.bass_utils as bu
    bu.upload_artifacts = lambda tmpdir: f"local://{tmpdir}"


def kernel(**inputs):
    global LAST_EXEC_NS
    inputs = {k: np.asarray(v) for k, v in inputs.items()}
    if not _NC_CACHE:
        _NC_CACHE.append(_build())
    nc = _NC_CACHE[0]
    in_maps = [_prep_inputs(inputs, core) for core in range(NCORES)]
    kwargs = {}
    if os.environ.get("DEC_TRACE") == "1":
        _install_trace_shim()
        import tempfile
        kwargs = dict(trace=True, tmpdir=tempfile.mkdtemp(prefix="dec_trace_"))
    res = run_bass_kernel_spmd(nc, in_maps, core_ids=list(range(NCORES)), **kwargs)
    LAST_EXEC_NS = res.exec_time_ns
    out = np.concatenate([res.results[c]["out_l"] for c in range(NCORES)], axis=0)
    return out.astype(np.float32)



# revision 14
# speedup vs baseline: 1.1793x; 1.0033x over previous
"""Trainium2 Bass kernel: GRU decoder with Luong attention (B=32, T=S=512, H=1024, D=80).

Strategy (8 NeuronCores, data-parallel over batch, 4 sequences per core):
  P0: gx = W_ih @ x precomputed for the n gate only (fp32r matmuls) -> DRAM.
      The r,z gate gx+bias are folded into the recurrent PSUM accumulation
      via an extra matmul per chunk against the augmented input x~=[x;1]
      (weights [W_ih_rz; b_rz], bf16, zero-padded to 128 partitions — the
      PE FWL weight loader corrupts sub-128-partition bf16 loads).
  P1: sequential GRU in transposed layout (H on partitions, batch on free).
      Per step: 16 h-independent gx matmuls (cover the previous step's gate
      tail), then 192 (LDWEIGHTS+MATMUL) pairs streaming W_hh.T bf16 tiles
      against the bf16 h vector. Gate sigmoids read PSUM directly (bias
      folded via x~), the bf16 h for the next step is written by DVE right
      after the z-multiply (same-engine FIFO), and the f32 h copy (catT,
      attention input) goes to GPSIMD off the critical path. Explicit
      scheduling deps pin the gx matmuls behind the previous step's last
      matmul and keep the ACT queue order [tanh, z-sigmoid] — the Tile
      scheduler's cost model underestimates the matmul stream and otherwise
      misorders both.
  P2: attention for all timesteps at once (scoresT -> softmax -> PE
      transpose -> ctxT), then the concat linear (tanh) and output linear
      as fp32r matmuls.

All per-core inputs are sliced/transposed on the host; the 8 cores run the
same NEFF via run_bass_kernel_spmd with per-core input maps.
"""

import os
import sys

for _p in ("/opt/trn_rl_repo", "/root/.axon_site/_ro/trn_rl_repo"):
    if os.path.isdir(_p) and _p not in sys.path:
        sys.path.insert(0, _p)

import numpy as np
import ml_dtypes

import concourse.bass as bass
import concourse.mybir as mybir
import concourse.tile as tile
from concourse import bacc
from concourse.bass_utils import run_bass_kernel_spmd
from concourse.masks import make_identity

dt = mybir.dt
AF = mybir.ActivationFunctionType
ALU = mybir.AluOpType

H, D, B, S, T = 1024, 80, 32, 512, 512
BL = 4          # batch per core
NCORES = 8
KC = 8          # H / 128
MC = 24         # 3H / 128
KC2 = 16        # 2H / 128
NC8 = 8         # n-gate chunk count


def _build():
    nc = bacc.Bacc("TRN2", target_bir_lowering=False, debug=False,
                   num_devices=NCORES)
    f32r = dt.float32r

    w_hhT = nc.dram_tensor("w_hhT", [H, 3 * H], dt.bfloat16, kind="ExternalInput")
    w_ihTn = nc.dram_tensor("w_ihTn", [D, H], dt.float32, kind="ExternalInput")
    xT = nc.dram_tensor("xT", [D, T * BL], dt.float32, kind="ExternalInput")
    xa = nc.dram_tensor("xa", [128, T * BL], dt.bfloat16, kind="ExternalInput")
    wrz = nc.dram_tensor("wrz", [128, 2 * H], dt.bfloat16, kind="ExternalInput")
    bias_nT = nc.dram_tensor("bias_nT", [128, NC8], dt.float32, kind="ExternalInput")
    h0T = nc.dram_tensor("h0T", [128, KC, BL], dt.float32, kind="ExternalInput")
    encT_d = nc.dram_tensor("encT_d", [KC, 128, BL, S], dt.float32, kind="ExternalInput")
    enc_d = nc.dram_tensor("enc_d", [4, 128, BL, H], dt.float32, kind="ExternalInput")
    w_cT = nc.dram_tensor("w_cT", [2 * H, H], dt.float32, kind="ExternalInput")
    b_cT = nc.dram_tensor("b_cT", [128, KC], dt.float32, kind="ExternalInput")
    w_oT = nc.dram_tensor("w_oT", [H, D], dt.float32, kind="ExternalInput")
    b_o_b = nc.dram_tensor("b_o_b", [128, D], dt.float32, kind="ExternalInput")
    maskTb = nc.dram_tensor("maskTb", [128, 4, BL], dt.float32, kind="ExternalInput")
    bhhn = nc.dram_tensor("bhhn", [128, KC, BL], dt.float32, kind="ExternalInput")

    out_l = nc.dram_tensor("out_l", [BL, T, D], dt.float32, kind="ExternalOutput")
    gxT_d = nc.dram_tensor("gxT_d", [NC8, 128, T * BL], dt.float32)

    with tile.TileContext(nc) as tc:
        with tc.tile_pool(name="persist", bufs=1) as persist:
            catT = persist.tile([128, KC2, T, BL], dt.float32r)
            ident_f = persist.tile([128, 128], dt.float32)
            make_identity(nc, ident_f)
            ident = persist.tile([128, 128], dt.float32r)
            nc.vector.tensor_copy(out=ident[:], in_=ident_f[:])

            # ---- P0: gx precompute (n gate only) ----
            with tc.tile_pool(name="p0", bufs=1) as p0, \
                 tc.tile_pool(name="p0o", bufs=4) as p0o, \
                 tc.tile_pool(name="psA", bufs=2, space="PSUM") as psA:
                biasn_sb = p0.tile([128, NC8], dt.float32)
                nc.sync.dma_start(out=biasn_sb, in_=bias_nT.ap())
                xT_sb = p0.tile([D, T * BL], dt.float32r)
                nc.sync.dma_start(out=xT_sb, in_=xT.ap().bitcast(f32r))
                wihn_sb = p0.tile([D, NC8, 128], dt.float32r)
                nc.sync.dma_start(
                    out=wihn_sb,
                    in_=w_ihTn.ap().bitcast(f32r).rearrange("p (mc m) -> p mc m", m=128))
                for mc in range(NC8):
                    for nt in range(4):
                        ps = psA.tile([128, 512], dt.float32, tag="gx")
                        nc.tensor.matmul(ps[:], wihn_sb[:, mc, :],
                                         xT_sb[:, nt * 512:(nt + 1) * 512],
                                         start=True, stop=True)
                        gxs = p0o.tile([128, 512], dt.float32, tag="gxo")
                        if (mc + nt) % 2 == 0:
                            nc.scalar.activation(out=gxs[:], in_=ps[:], func=AF.Identity,
                                                 bias=biasn_sb[:, mc:mc + 1], scale=1.0)
                        else:
                            nc.vector.tensor_scalar_add(gxs[:], ps[:],
                                                        biasn_sb[:, mc:mc + 1])
                        nc.sync.dma_start(out=gxT_d.ap()[mc, :, nt * 512:(nt + 1) * 512],
                                          in_=gxs[:])

            # ---- P1: sequential GRU ----
            with tc.tile_pool(name="p1w", bufs=1) as p1w, \
                 tc.tile_pool(name="p1", bufs=1) as p1, \
                 tc.tile_pool(name="gxc", bufs=2) as gxcp, \
                 tc.tile_pool(name="p1t", bufs=3) as p1t, \
                 tc.tile_pool(name="psG", bufs=2, space="PSUM") as psG:
                w_sb = p1w.tile([128, KC, MC, 128], dt.bfloat16)
                for kc in range(KC):
                    nc.sync.dma_start(
                        out=w_sb[:, kc, :, :],
                        in_=w_hhT.ap()[kc * 128:(kc + 1) * 128, :]
                            .rearrange("p (mc m) -> p mc m", m=128))
                xa_sb = p1.tile([128, T * BL], dt.bfloat16)
                nc.sync.dma_start(out=xa_sb, in_=xa.ap())
                wrz_sb = p1.tile([128, 16, 128], dt.bfloat16)
                nc.sync.dma_start(
                    out=wrz_sb,
                    in_=wrz.ap().rearrange("p (mc m) -> p mc m", m=128))
                h_bf = p1.tile([128, 2, KC, BL], dt.bfloat16)
                h0_sb = p1.tile([128, KC, BL], dt.float32)
                nc.sync.dma_start(out=h0_sb, in_=h0T.ap())
                bhhn_sb = p1.tile([128, KC, BL], dt.float32)
                nc.sync.dma_start(out=bhhn_sb, in_=bhhn.ap())
                nc.vector.tensor_copy(out=h_bf[:, 0, :, :], in_=h0_sb[:])

                CH = 16
                gx_chunks = []
                for c in range(T // CH):
                    gxc = gxcp.tile([128, NC8, CH * BL], dt.float32, tag="gxc")
                    nc.sync.dma_start(out=gxc,
                                      in_=gxT_d.ap().rearrange("mc p c -> p mc c")
                                      [:, :, c * CH * BL:(c + 1) * CH * BL])
                    gx_chunks.append(gxc)

                i_last_mm = None
                for t in range(T):
                    c, j = divmod(t, CH)
                    h2 = h_bf[:, t % 2, :, :]
                    g_r = psG.tile([128, KC, BL], dt.float32, tag="gr")
                    g_n = psG.tile([128, KC, BL], dt.float32, tag="gn")
                    g_z = psG.tile([128, KC, BL], dt.float32, tag="gz")
                    xa_t = xa_sb[:, t * BL:(t + 1) * BL]
                    # h-independent gx matmuls first: cover the previous tail.
                    # start=True only on the first write per PSUM bank (a
                    # start matmul clears has_written for the whole bank).
                    # Pinned after the previous step's last matmul so the
                    # scheduler's PE order matches emission order.
                    for mc in range(KC):
                        i_gx = nc.tensor.matmul(g_r[:, mc, :],
                                                wrz_sb[:, mc, :], xa_t,
                                                start=(mc == 0), stop=False,
                                                skip_group_check=True)
                        if i_last_mm is not None:
                            tile.add_dep_helper(i_gx.ins, i_last_mm.ins,
                                                sync=True,
                                                reason="pin gx after prev step")
                    for mc in range(KC):
                        i_gx = nc.tensor.matmul(g_z[:, mc, :],
                                                wrz_sb[:, KC + mc, :], xa_t,
                                                start=(mc == 0), stop=False,
                                                skip_group_check=True)
                        if i_last_mm is not None:
                            tile.add_dep_helper(i_gx.ins, i_last_mm.ins,
                                                sync=True,
                                                reason="pin gx after prev step")
                    # r gate W_hh matmuls
                    for mc in range(KC):
                        for kc in range(KC):
                            nc.tensor.matmul(g_r[:, mc, :], w_sb[:, kc, mc, :],
                                             h2[:, kc, :],
                                             start=False, stop=(kc == KC - 1),
                                             skip_group_check=True)
                    r_s = p1t.tile([128, KC, BL], dt.float32, tag="r_s")
                    nc.scalar.activation(out=r_s[:], in_=g_r[:], func=AF.Sigmoid)
                    # n gate W_hh matmuls
                    for mc in range(16, MC):
                        for kc in range(KC):
                            nc.tensor.matmul(g_n[:, mc - 16, :], w_sb[:, kc, mc, :],
                                             h2[:, kc, :],
                                             start=(kc == 0), stop=(kc == KC - 1))
                    gxt = gx_chunks[c][:, :, j * BL:(j + 1) * BL]
                    hold = h0_sb[:] if t == 0 else catT[:, 0:KC, t - 1, :]
                    tn = p1t.tile([128, KC, BL], dt.float32, tag="tn")
                    nc.vector.tensor_add(tn[:], g_n[:], bhhn_sb[:])
                    nc.vector.tensor_mul(tn[:], tn[:], r_s[:])
                    nc.vector.tensor_add(tn[:], tn[:], gxt[:])
                    i_tanh = nc.scalar.activation(out=tn[:], in_=tn[:], func=AF.Tanh)
                    tu = p1t.tile([128, KC, BL], dt.float32, tag="tu")
                    nc.vector.tensor_sub(tu[:], hold, tn[:])
                    # z gate W_hh matmuls
                    for mc in range(KC, 16):
                        for kc in range(KC):
                            i_last_mm = nc.tensor.matmul(
                                g_z[:, mc - KC, :], w_sb[:, kc, mc, :],
                                h2[:, kc, :],
                                start=False, stop=(kc == KC - 1),
                                skip_group_check=True)
                    z_s = p1t.tile([128, KC, BL], dt.float32, tag="z_s")
                    i_zsig = nc.scalar.activation(out=z_s[:], in_=g_z[:],
                                                  func=AF.Sigmoid)
                    # keep ACT FIFO order [.., tanh, z-sig]
                    tile.add_dep_helper(i_zsig.ins, i_tanh.ins, sync=True,
                                        reason="ACT order: tanh before z-sig")
                    nc.vector.tensor_mul(z_s[:], z_s[:], tu[:])
                    # bf16 h on DVE (same-engine FIFO after the mul)
                    nc.vector.tensor_add(h_bf[:, (t + 1) % 2, :, :], z_s[:], tn[:])
                    # f32 catT on POOL (off critical path)
                    nc.gpsimd.tensor_add(catT[:, 0:KC, t, :], z_s[:], tn[:])

            # ---- P2a: attention ----
            with tc.tile_pool(name="p2a", bufs=1) as p2a, \
                 tc.tile_pool(name="p2aT", bufs=2) as p2aT, \
                 tc.tile_pool(name="p2at", bufs=2) as p2at, \
                 tc.tile_pool(name="p2t", bufs=4) as p2t, \
                 tc.tile_pool(name="psB", bufs=2, space="PSUM") as psB:
                for b in range(BL):
                    encTb = p2aT.tile([128, KC, S], dt.float32r, tag="encT")
                    nc.sync.dma_start(out=encTb, in_=encT_d.ap().bitcast(dt.float32r)
                                      .rearrange("kc p b s -> p kc b s")[:, :, b, :])
                    encNb = p2a.tile([128, 4, H], dt.float32r, tag="encN")
                    nc.sync.dma_start(out=encNb, in_=enc_d.ap().bitcast(dt.float32r)
                                      .rearrange("sc p b h -> p sc b h")[:, :, b, :])
                    attnT = p2at.tile([128, 4, T], dt.float32r, tag="attnT")
                    for tcn in range(T // 128):
                        ps_sc = psB.tile([128, S], dt.float32, tag="sc")
                        for kc in range(KC):
                            nc.tensor.matmul(ps_sc[:],
                                             catT[:, kc, tcn * 128:(tcn + 1) * 128, b],
                                             encTb[:, kc, :],
                                             start=(kc == 0), stop=(kc == KC - 1))
                        negmax = p2t.tile([128, 1], dt.float32, tag="mx")
                        nc.vector.tensor_reduce(negmax[:], ps_sc[:],
                                                axis=mybir.AxisListType.X,
                                                op=ALU.max, negate=True)
                        attn = p2t.tile([128, S], dt.float32r, tag="attn")
                        ssum = p2t.tile([128, 1], dt.float32, tag="ssum")
                        nc.scalar.activation(out=attn[:], in_=ps_sc[:], func=AF.Exp,
                                             bias=negmax[:], scale=1.0,
                                             accum_out=ssum[:])
                        rinv = p2t.tile([128, 1], dt.float32, tag="rinv")
                        nc.vector.reciprocal(rinv[:], ssum[:])
                        nc.vector.tensor_scalar_mul(attn[:], attn[:], rinv[:])
                        for sc in range(4):
                            ps_tr = psB.tile([128, 128], dt.float32r, tag="tr")
                            nc.tensor.transpose(ps_tr[:],
                                                attn[:, sc * 128:(sc + 1) * 128],
                                                ident[:])
                            nc.vector.tensor_copy(
                                out=attnT[:, sc, tcn * 128:(tcn + 1) * 128],
                                in_=ps_tr[:])
                    for hc in range(KC):
                        ps_ctx = psB.tile([128, T], dt.float32, tag="ctx")
                        for sc in range(4):
                            nc.tensor.matmul(ps_ctx[:],
                                             encNb[:, sc, hc * 128:(hc + 1) * 128],
                                             attnT[:, sc, :],
                                             start=(sc == 0), stop=(sc == 3))
                        nc.vector.tensor_copy(out=catT[:, KC + hc, :, b], in_=ps_ctx[:])

            # ---- P2b: concat linear + out linear ----
            with tc.tile_pool(name="p2b", bufs=1) as p2b, \
                 tc.tile_pool(name="wc", bufs=6) as wcp, \
                 tc.tile_pool(name="cT", bufs=4) as cTp, \
                 tc.tile_pool(name="p2o", bufs=4) as p2o, \
                 tc.tile_pool(name="psC", bufs=1, space="PSUM") as psC, \
                 tc.tile_pool(name="psC2", bufs=2, space="PSUM") as psC2:
                bc_sb = p2b.tile([128, KC], dt.float32)
                nc.sync.dma_start(out=bc_sb, in_=b_cT.ap())
                wo_sb = p2b.tile([128, KC, D], dt.float32r)
                nc.sync.dma_start(out=wo_sb, in_=w_oT.ap().bitcast(dt.float32r)
                                  .rearrange("(kc p) d -> p kc d", p=128))
                bo_sb = p2b.tile([128, D], dt.float32)
                nc.sync.dma_start(out=bo_sb, in_=b_o_b.ap())
                mask_sb = p2b.tile([128, 4, BL], dt.float32)
                nc.sync.dma_start(out=mask_sb, in_=maskTb.ap())
                wcT_ap = w_cT.ap().bitcast(dt.float32r).rearrange(
                    "(kc p) (mc m) -> p kc mc m", p=128, m=128)
                # concat linear: one W_c tile load serves all 4 batches
                # (4x less DMA, denser PE stream than per-batch streaming)
                cTbs = [cTp.tile([128, KC, T], dt.float32r, tag="cT",
                                 name=f"cTb{b}") for b in range(BL)]
                for mc2 in range(KC):
                    pss = [psC.tile([128, T], dt.float32, tag=f"c{b}",
                                    name=f"psc{b}") for b in range(BL)]
                    for kc2 in range(KC2):
                        wt = wcp.tile([128, 128], dt.float32r, tag="wc")
                        nc.sync.dma_start(out=wt, in_=wcT_ap[:, kc2, mc2, :])
                        for b in range(BL):
                            nc.tensor.matmul(pss[b][:], wt[:], catT[:, kc2, :, b],
                                             start=(kc2 == 0),
                                             stop=(kc2 == KC2 - 1))
                    for b in range(BL):
                        nc.scalar.activation(out=cTbs[b][:, mc2, :], in_=pss[b][:],
                                             func=AF.Tanh,
                                             bias=bc_sb[:, mc2:mc2 + 1], scale=1.0)
                for b in range(BL):
                    cTb = cTbs[b]
                    for tcn in range(T // 128):
                        ps_o = psC2.tile([128, D], dt.float32, tag="o")
                        for hc in range(KC):
                            nc.tensor.matmul(ps_o[:],
                                             cTb[:, hc, tcn * 128:(tcn + 1) * 128],
                                             wo_sb[:, hc, :],
                                             start=(hc == 0), stop=(hc == KC - 1))
                        o_sb = p2o.tile([128, D], dt.float32, tag="o_s")
                        nc.vector.tensor_add(o_sb[:], ps_o[:], bo_sb[:])
                        nc.vector.tensor_scalar_mul(o_sb[:], o_sb[:],
                                                    mask_sb[:, tcn, b:b + 1])
                        nc.sync.dma_start(
                            out=out_l.ap()[b, tcn * 128:(tcn + 1) * 128, :],
                            in_=o_sb[:])

    nc.compile()
    return nc


def _prep_inputs(inputs, core):
    boff = core * BL
    enc = np.ascontiguousarray(inputs["encoder_outputs"][boff:boff + BL])
    tgt = inputs["target_tensor"][boff:boff + BL]
    tl = inputs["target_length"][boff:boff + BL]
    h0 = inputs["h0"][0, boff:boff + BL]
    W_ih, W_hh = inputs["W_ih"], inputs["W_hh"]
    b_g = (inputs["b_ih"] + inputs["b_hh"]).astype(np.float32)
    b_g[2 * H:] = inputs["b_ih"][2 * H:]   # b_hh_n goes inside the r-multiply
    bhhn_np = np.broadcast_to(
        inputs["b_hh"][2 * H:].astype(np.float32)
        .reshape(KC, 128).T[:, :, None], (128, KC, BL)).copy()

    xs = np.concatenate([np.zeros((1, BL, D), np.float32),
                         tgt.transpose(1, 0, 2)[:-1]], 0)
    xT = np.ascontiguousarray(xs.reshape(T * BL, D).T)
    xa_np = np.zeros((128, T * BL), np.float32)
    xa_np[:D] = xT
    xa_np[D] = 1.0
    wrz_np = np.zeros((128, 2 * H), np.float32)
    wrz_np[:D] = W_ih.T[:, :2 * H]
    wrz_np[D] = b_g[:2 * H]

    return {
        "w_hhT": np.ascontiguousarray(W_hh.T).astype(ml_dtypes.bfloat16),
        "w_ihTn": np.ascontiguousarray(W_ih.T[:, 2 * H:]).astype(np.float32),
        "xT": xT.astype(np.float32),
        "xa": xa_np.astype(ml_dtypes.bfloat16),
        "wrz": wrz_np.astype(ml_dtypes.bfloat16),
        "bias_nT": np.ascontiguousarray(
            b_g[2 * H:].reshape(NC8, 128).T).astype(np.float32),
        "h0T": np.ascontiguousarray(h0.T.reshape(KC, 128, BL).transpose(1, 0, 2)),
        "encT_d": np.ascontiguousarray(
            enc.transpose(2, 1, 0).reshape(KC, 128, S, BL).transpose(0, 1, 3, 2)),
        "enc_d": np.ascontiguousarray(enc.transpose(1, 0, 2).reshape(4, 128, BL, H)),
        "w_cT": np.ascontiguousarray(inputs["W_c"].T).astype(np.float32),
        "b_cT": np.ascontiguousarray(inputs["b_c"].reshape(KC, 128).T),
        "w_oT": np.ascontiguousarray(inputs["W_o"].T),
        "b_o_b": np.broadcast_to(inputs["b_o"], (128, D)).copy(),
        "maskTb": np.ascontiguousarray(
            (np.arange(T)[:, None] < tl[None, :]).astype(np.float32)
            .reshape(4, 128, BL).transpose(1, 0, 2)),
        "bhhn": bhhn_np,
    }


_NC_CACHE = []
LAST_EXEC_NS = None


def _install_trace_shim():
    """antenv.axon_hooks shim so trace=True works under axon in this container."""
    import types, ctypes, contextlib
    if "antenv.axon_hooks" in sys.modules:
        return
    so_path = "/opt/axon/libaxon_pjrt.so"
    hook = None
    if os.path.exists(so_path):
        lib = ctypes.CDLL(so_path)
        if hasattr(lib, "axon_start_nrt_profile"):
            lib.axon_start_nrt_profile.argtypes = [ctypes.POINTER(ctypes.c_int64),
                                                   ctypes.c_size_t]
            lib.axon_start_nrt_profile.restype = ctypes.c_int64
            lib.axon_stop_nrt_profile.argtypes = [ctypes.c_char_p]
            lib.axon_stop_nrt_profile.restype = ctypes.c_int64

            @contextlib.contextmanager
            def _hook(output_dir, device_ids):
                import jax
                jax.devices()
                if device_ids:
                    ids = (ctypes.c_int64 * len(device_ids))(*device_ids)
                    rc = lib.axon_start_nrt_profile(ids, len(device_ids))
                else:
                    rc = lib.axon_start_nrt_profile(None, 0)
                if rc != 0:
                    raise RuntimeError(f"axon_start_nrt_profile rc={rc}")
                try:
                    yield
                finally:
                    n = lib.axon_stop_nrt_profile(str(output_dir).encode())
                    print(f"profile: {n} file(s) written to {output_dir}",
                          file=sys.stderr)
            hook = _hook
    mod = types.ModuleType("antenv.axon_hooks")
    mod.get_axon_ntff_profile_hook = lambda: hook
    mod.set_axon_ntff_profile_hook = lambda h: None
    sys.modules["antenv.axon_hooks"] = mod
    import concourse.bass_utils as bu
    bu.upload_artifacts = lambda tmpdir: f"local://{tmpdir}"


def kernel(**inputs):
    global LAST_EXEC_NS
    inputs = {k: np.asarray(v) for k, v in inputs.items()}
    if not _NC_CACHE:
        _NC_CACHE.append(_build())
    nc = _NC_CACHE[0]
    in_maps = [_prep_inputs(inputs, core) for core in range(NCORES)]
    kwargs = {}
    if os.environ.get("DEC_TRACE") == "1":
        _install_trace_shim()
        import tempfile
        kwargs = dict(trace=True, tmpdir=tempfile.mkdtemp(prefix="dec_trace_"))
    res = run_bass_kernel_spmd(nc, in_maps, core_ids=list(range(NCORES)), **kwargs)
    LAST_EXEC_NS = res.exec_time_ns
    out = np.concatenate([res.results[c]["out_l"] for c in range(NCORES)], axis=0)
    return out.astype(np.float32)

